# revision 7
# baseline (speedup 1.0000x reference)
"""Conv2d 3x3 (stride 1, pad 1) as implicit GEMM on 8 Trainium2 NeuronCores.

Problem: x[32,128,56,56] f32, weights[128,128,3,3] f32, bias[128] f32
         -> out[32,128,56,56] f32.

Sharding: data-parallel over batch — 4 images per core, weights/bias
replicated on every core.

Per-core kernel design (v3):
  - channels (128) live on the SBUF partition dim.
  - the host pre-pads each image into a flat row layout
      [margin 58 | 56 rows x (56 data + 1 zero) | margin 58]
    so every conv tap (kh,kw) is a plain shifted window d=(kh-1)*57+(kw-1)
    of one flat buffer, and the device DMA is fully contiguous.
  - x / weights are cast to bf16 on the host (PSUM still accumulates
    fp32) halving input DMA bytes and weight-load time; output is
    evicted as bf16 and upcast on the host.  bf16 matmuls back-to-back
    into one PSUM bank run at ~193ns/456-col matmul (the 1 col/cycle
    floor); NOTE: interleaving PSUM banks between consecutive matmuls
    (tap-outer order) costs ~40ns/matmul extra — do not do that.
  - per group: 9 tap matmuls accumulate into one PSUM bank, then the
    scalar engine evicts PSUM->SBUF with fused bias (dropping the pad
    columns) and the output DMA is triggered from the sync engine.
  - the last image's final 8 rows are split into two 4-row groups so the
    tail (last evict + DMA) is short.
  - head latency discipline: image-0's first chunk is DMA'd via the
    scalar engine (its instruction stream starts ~1us before sync's),
    the first weight chunk is sync's first trigger, and all later input
    transfers are gated on compute milestones so their descriptors do
    not clog the DMA queues ahead of the first chunk (the queues also
    carry the engines' instruction-stream fetches during the first
    ~10us, so a free-for-all reliably starves the critical chunk).
  - a few bf16 matmuls on a zero scratch warm the PE clock (HAM, ~3us
    to full speed) during the framework preamble / first DMA wait.
"""

import numpy as np

N_TOTAL = 32
N_CORES = 8
N_PER_CORE = N_TOTAL // N_CORES
C = 128
H = W = 56
HW = H * W            # 3136
WP = W + 1            # 57  padded row width (shared pad col)
L = H * WP            # 3192 flat padded length
MARGIN = WP + 1       # 58  covers worst tap offset
TILE_W = MARGIN + L + MARGIN  # 3308
GW = 8 * WP           # 456 (<=512 fp32 PSUM bank)
N_WARM = 6
# image-0 chunk bounds: fine-grained so the first groups' deps are small
X_BOUNDS0 = [0, MARGIN + GW + MARGIN, MARGIN + 2 * GW + MARGIN,
             MARGIN + 3 * GW + MARGIN, MARGIN + 5 * GW + MARGIN, TILE_W]
# images 1-3: two chunks
XA = MARGIN + 4 * GW + MARGIN  # 1940

# (image, row0, nrows) for every PSUM group; the last image ends with two
# 4-row groups to shorten the tail.
GROUPS = []
for _n in range(N_PER_CORE):
    if _n < N_PER_CORE - 1:
        GROUPS += [(_n, r, 8) for r in range(0, H, 8)]
    else:
        GROUPS += [(_n, r, 8) for r in range(0, 48, 8)]
        GROUPS += [(_n, 48, 6), (_n, 54, 2)]

_CACHE = {}


def _build_nc():
    import concourse.mybir as mybir
    import concourse.tile as tile
    from concourse import bacc
    from concourse.tile import add_dep_helper

    f32 = mybir.dt.float32
    bf16 = mybir.dt.bfloat16
    af = mybir.ActivationFunctionType

    nc = bacc.Bacc("TRN2", target_bir_lowering=False, debug=False)

    x_d = nc.dram_tensor("x", [N_PER_CORE, C, TILE_W], bf16, kind="ExternalInput")
    w_d = nc.dram_tensor("w", [C, 9 * C], bf16, kind="ExternalInput")
    b_d = nc.dram_tensor("b", [C, 1], f32, kind="ExternalInput")
    y_d = nc.dram_tensor("y", [N_PER_CORE, C, HW], bf16, kind="ExternalOutput")

    with tile.TileContext(nc) as tc:
        with (
            tc.tile_pool(name="const", bufs=1) as cpool,
            tc.tile_pool(name="xbuf", bufs=1) as xpool,
            tc.tile_pool(name="obuf", bufs=2) as opool,
            tc.tile_pool(name="psum", bufs=8, space="PSUM") as ppool,
        ):
            # PE warm-up on a zero scratch (HAM clock ramp), started as
            # early as possible: gpsimd does the memset because its
            # instruction stream starts earliest.
            zsc = cpool.tile([C, GW], bf16, tag="zsc")
            nc.gpsimd.memset(zsc[:], 0.0)
            for _ in range(N_WARM):
                wm = ppool.tile([C, GW], f32, tag="ps", name="ps")
                nc.tensor.matmul(wm[:], zsc[:, 0:C], zsc[:], start=True, stop=True)

            xts = [xpool.tile([C, TILE_W], bf16, tag=f"x{n}", name=f"x{n}")
                   for n in range(N_PER_CORE)]
            wt = cpool.tile([C, 9 * C], bf16, tag="wt")
            bt = cpool.tile([C, 1], f32, tag="bt")

            # critical first transfers: x0 chunk0 on scalar (earlier
            # stream start), first weight chunk as sync's first trigger
            nc.scalar.dma_start(
                out=xts[0][:, 0 : X_BOUNDS0[1]], in_=x_d[0][:, 0 : X_BOUNDS0[1]]
            )
            for k in range(3):
                nc.sync.dma_start(
                    out=wt[:, 3 * k * C : 3 * (k + 1) * C],
                    in_=w_d[:, 3 * k * C : 3 * (k + 1) * C],
                )
            x_dmas = {0: [], 1: [], 2: [], 3: []}
            # x0 c1a/c1b ungated (needed by groups 1-2)
            for a, b in zip(X_BOUNDS0[1:3], X_BOUNDS0[2:4]):
                nc.sync.dma_start(out=xts[0][:, a:b], in_=x_d[0][:, a:b])
            nc.sync.dma_start(out=bt[:], in_=b_d[:])
            # tail of image 0 (gated on compute below)
            for a, b in zip(X_BOUNDS0[3:], X_BOUNDS0[4:]):
                x_dmas[0].append(
                    nc.sync.dma_start(out=xts[0][:, a:b], in_=x_d[0][:, a:b])
                )
            # images 1-3, two chunks each (gated on compute below)
            for n in range(1, N_PER_CORE):
                x_dmas[n].append(nc.sync.dma_start(
                    out=xts[n][:, 0:XA], in_=x_d[n][:, 0:XA]))
                x_dmas[n].append(nc.sync.dma_start(
                    out=xts[n][:, XA:TILE_W], in_=x_d[n][:, XA:TILE_W]))

            ots = {}
            gate_mms = {}   # image -> last matmul of its group 0
            gate2_mms = {}  # image -> last matmul of its group 2
            for n, row0, nrows in GROUPS:
                if n not in ots:
                    ots[n] = opool.tile([C, HW], bf16, tag="ot", name=f"ot{n}")
                ot = ots[n]
                width = nrows * WP
                ps = ppool.tile([C, GW], f32, tag="ps", name="ps")
                for t in range(9):
                    kh, kw = divmod(t, 3)
                    d = (kh - 1) * WP + (kw - 1)
                    base = MARGIN + row0 * WP + d
                    mm = nc.tensor.matmul(
                        ps[:, 0:width], wt[:, t * C : (t + 1) * C],
                        xts[n][:, base : base + width],
                        start=(t == 0), stop=(t == 8),
                    )
                if row0 == 0:
                    gate_mms[n] = mm
                if row0 == 16:
                    gate2_mms[n] = mm
                ni = nrows * W
                src = ps[:, 0:width]
                src = src.rearrange("p (r c) -> p r c", c=WP)[:, :, 0:W]
                dstp = ot[:, row0 * W : row0 * W + ni]
                dstp = dstp.rearrange("p (r c) -> p r c", c=W)
                nc.scalar.activation(dstp, src, af.Identity, bias=bt[:])
                nc.sync.dma_start(
                    out=y_d[n][:, row0 * W : row0 * W + ni],
                    in_=ot[:, row0 * W : row0 * W + ni],
                )

            # stagger the bulk input transfers behind compute milestones
            # so they don't contend with the critical head transfers
            for dma in x_dmas[0]:
                add_dep_helper(
                    dma.ins, gate_mms[0].ins, sync=True,
                    reason="stagger x0 tail",
                )
            for n in range(1, N_PER_CORE):
                for dma in x_dmas[n]:
                    add_dep_helper(
                        dma.ins, gate2_mms[n - 1].ins, sync=True,
                        reason="stagger x prefetch",
                    )

    nc.compile()
    return nc


def _get_nc():
    if "nc" not in _CACHE:
        _CACHE["nc"] = _build_nc()
    return _CACHE["nc"]


def _prep_inputs(x, weights, bias):
    import ml_dtypes

    bf16 = ml_dtypes.bfloat16
    x = np.asarray(x, dtype=np.float32).reshape(N_TOTAL, C, H, W)
    xp = np.zeros((N_TOTAL, C, TILE_W), dtype=bf16)
    # interior: rows of [56 data | 0], flat at offset MARGIN
    v = xp[:, :, MARGIN : MARGIN + L].reshape(N_TOTAL, C, H, WP)
    v[:, :, :, 0:W] = x.astype(bf16)
    # weights [co, ci, kh, kw] -> [ci, (kh kw), co] so each tap slice is a
    # contiguous [K=ci, M=co] lhsT tile
    w = np.asarray(weights, dtype=np.float32)
    w = np.ascontiguousarray(
        np.transpose(w, (1, 2, 3, 0)).reshape(C, 9 * C)
    ).astype(bf16)
    b = np.ascontiguousarray(np.asarray(bias, dtype=np.float32).reshape(C, 1))
    return xp, w, b


def kernel(x, weights, bias, _trace=False):
    from concourse.bass_utils import run_bass_kernel_spmd

    nc = _get_nc()
    xp, w, b = _prep_inputs(x, weights, bias)
    in_maps = [
        {"x": xp[i * N_PER_CORE : (i + 1) * N_PER_CORE], "w": w, "b": b}
        for i in range(N_CORES)
    ]
    res = run_bass_kernel_spmd(
        nc, in_maps, core_ids=list(range(N_CORES)), trace=_trace
    )
    y = np.concatenate([r["y"] for r in res.results], axis=0)
    y = y.astype(np.float32).reshape(N_TOTAL, C, H, W)
    if _trace:
        return y, res
    return y
